# revision 37
# baseline (speedup 1.0000x reference)
"""YOLO-v1-style loss on 8 Trainium2 NeuronCores (Bass/Tile).

Data-parallel over batch: each core gets 2048 of 16384 batch elements
(100,352 cells as 128 partitions x 784 cells); per-partition partial sums
for the 5 loss terms are combined on the host.

Layout: host repacks channels into per-channel planes ([P, plane, cells])
so DVE tensor_tensor ops run dense step-1 bf16 at 2x mode and
tensor_scalar ops at 4x. scalar_tensor_tensor (1x only) is avoided.

IoU via the overlap identity (no corner materialization):
  overlap_x = min(3.5*(pw+tw) - |px-tx|, 7*min(pw,tw)), clamped at 0.

Engine split:
  - DVE: box pipeline (2x-mode bf16 tensor_tensor) + obj/resp masks
  - ACT: sqrt/abs, square+accumulate reduction passes
  - Pool: SWDGE descriptor generation only (its tensor ops are slow and
    fight DVE for SBUF ports)
  - DMA CCE: class diff (p - t) computed inline in the DMA engines:
    p streams in with an fp8->bf16 cast, host-negated t accumulates with
    cce add. CCE accumulates silently corrupt beyond 2048 elements per
    partition per DMA, so the accumulate is issued in <=1960-element calls.

add_dep_helper sync=False edges steer the Tile list scheduler: the class
masks slot into the box chain where their DMA data has landed, and the
class stream is throttled behind the box stream for HBM priority.

Self-contained: hardcodes all shapes; needs numpy + ml_dtypes + concourse.
"""

import numpy as np
import ml_dtypes

import concourse.bass as bass
import concourse.bacc as bacc
import concourse.tile as tile
import concourse.mybir as mybir
from concourse.bass_utils import run_bass_kernel_spmd
from bass_rust import add_dep_helper

f32 = mybir.dt.float32
bf16 = mybir.dt.bfloat16
f8e3 = mybir.dt.float8e3
Alu = mybir.AluOpType
Act = mybir.ActivationFunctionType

S = 7
BATCH = 16384
NCORES = 8
PER = BATCH // NCORES          # 2048 batch elems per core
P = 128                        # partitions
F = PER * S * S // P           # 784 cells per partition
NCQ = 4                        # class cell chunks
MQ = F // NCQ                  # 196
HS = S / 2.0                   # 3.5
S2 = float(S * S)              # 49

NACC = 4 + NCQ + 2


def _bc(x, r):
    """[P, ...] -> [P, r, ...]: broadcast (step-0) over a new outer dim."""
    return bass.AP(tensor=x.tensor, offset=x.offset,
                   ap=[x.ap[0], [0, r]] + list(x.ap[1:]))


def build_nc():
    nc = bacc.Bacc("TRN2", target_bir_lowering=False, debug=False,
                   num_devices=NCORES)
    # box planes (bf16) [P, 19, F]:
    #   0-3  X4 = px0 px1 tx0 tx1      4-7  W4 = pw0 pw1 tw0 tw1
    #   8-11 Y4 = py0 py1 ty0 ty1     12-15 H4 = ph0 ph1 th0 th1
    #   16-18 C3 = pc0 pc1 conf
    box = nc.dram_tensor("box", [P, 19, F], bf16, kind="ExternalInput")
    # class streams (fp8 e3m4): pred and negated target, channel-major
    cvp = nc.dram_tensor("cvp", [NCQ, P, 20, MQ], f8e3, kind="ExternalInput")
    cvn = nc.dram_tensor("cvn", [NCQ, P, 20, MQ], f8e3, kind="ExternalInput")

    out = nc.dram_tensor("acc_out", [P, NACC], f32, kind="ExternalOutput")

    V = nc.vector
    A = nc.scalar
    G = nc.gpsimd

    with tile.TileContext(nc) as tc:
        with (
            tc.tile_pool(name="inp", bufs=1) as inp,
            tc.tile_pool(name="cls", bufs=4) as clsb,
            tc.tile_pool(name="wk", bufs=1) as wk,
            tc.tile_pool(name="one", bufs=1) as one,
        ):
            # no memset: every acc column is written exactly once via accum_out
            acc = one.tile([P, NACC], f32)

            # ---- box DMAs first: V work starts as soon as X4 lands ----
            bxa = inp.tile([P, 8, F], bf16, tag="bxa")   # X4 W4
            nc.sync.dma_start(bxa[:, 0:4], box.ap()[:, 0:4])
            i_w4 = nc.sync.dma_start(bxa[:, 4:8], box.ap()[:, 4:8])
            bxb = inp.tile([P, 8, F], bf16, tag="bxb")   # Y4 H4
            nc.sync.dma_start(bxb[:, 0:4], box.ap()[:, 8:12])
            i_h4 = nc.sync.dma_start(bxb[:, 4:8], box.ap()[:, 12:16])
            bxc = inp.tile([P, 3, F], bf16, tag="bxc")   # C3
            i_c3 = nc.sync.dma_start(bxc, box.ap()[:, 16:19])
            box_gate = [i_w4, i_h4, i_c3, i_c3]

            # ---- class diff stream: SWDGE cast + CCE add (d = p - t) ----
            # CCE accumulate caps at 2048 elems/partition per DMA: the
            # cast moves all 20 channels, the accumulate goes in 2 halves.
            cv = []
            for q in range(NCQ):
                dv = clsb.tile([P, 20, MQ], bf16, tag="dv")
                ic = G.dma_start(dv, cvp.ap()[q])
                # throttle class stream behind the box DMAs so the box
                # pipeline's inputs land at full HBM rate first (q0 is free:
                # it overlaps the NEFF preamble window)
                if q > 0:
                    add_dep_helper(ic.ins, box_gate[q].ins,
                                   reason="cls stream after box stream")
                G.dma_start(dv[:, 0:10], cvn.ap()[q][:, 0:10],
                            accum_op=Alu.add)
                G.dma_start(dv[:, 10:20], cvn.ap()[q][:, 10:20],
                            accum_op=Alu.add)
                cv.append(dv)

            X, W = bxa[:, 0:4], bxa[:, 4:8]
            Y, H = bxb[:, 0:4], bxb[:, 4:8]
            pc = bxc[:, 0:2]
            conf = bxc[:, 2]

            # squared-loss ingredients [dx(2) dy(2) dw(2) dh(2) do(2) nb(2)]
            dsq = wk.tile([P, 6, 2, F], bf16, tag="dsq")

            # dx_b = px_b - tx0 (slot-0 target; dsq[0,1] re-done per slot later)
            V.tensor_tensor(dsq[:, 0], X[:, 0:2], _bc(X[:, 2], 2),
                            op=Alu.subtract)
            V.tensor_tensor(dsq[:, 1], Y[:, 0:2], _bc(Y[:, 2], 2),
                            op=Alu.subtract)
            adx = wk.tile([P, 2, 2, F], bf16, tag="adx")
            A.activation(adx[:, 0], dsq[:, 0], Act.Abs)
            A.activation(adx[:, 1], dsq[:, 1], Act.Abs)

            # u = 3.5*(pw_b + tw0) - |dx|
            u = wk.tile([P, 2, 2, F], bf16, tag="u")
            V.tensor_tensor(u[:, 0], W[:, 0:2], _bc(W[:, 2], 2), op=Alu.add)
            V.tensor_tensor(u[:, 1], H[:, 0:2], _bc(H[:, 2], 2), op=Alu.add)
            V.tensor_scalar(u, u, HS, None, op0=Alu.mult)
            V.tensor_tensor(u, u, adx, op=Alu.subtract)

            # m7 = 7 * min(pw_b, tw0); overlap = relu(min(u, m7))
            m7 = wk.tile([P, 2, 2, F], bf16, tag="m7")
            V.tensor_tensor(m7[:, 0], W[:, 0:2], _bc(W[:, 2], 2), op=Alu.min)
            V.tensor_tensor(m7[:, 1], H[:, 0:2], _bc(H[:, 2], 2), op=Alu.min)
            V.tensor_scalar(m7, m7, float(S), None, op0=Alu.mult)
            whr = wk.tile([P, 2, 2, F], bf16, tag="whr")
            V.tensor_tensor(whr, u, m7, op=Alu.min)
            V.tensor_scalar(whr, whr, 0.0, None, op0=Alu.max)

            inter = wk.tile([P, 2, F], bf16, tag="inter")
            V.tensor_tensor(inter, whr[:, 0], whr[:, 1], op=Alu.mult)

            # areas (x S^2): [ap0 ap1 at]
            ar = wk.tile([P, 3, F], bf16, tag="ar")
            V.tensor_scalar(ar, W[:, 0:3], S2, None, op0=Alu.mult)
            V.tensor_tensor(ar, ar, H[:, 0:3], op=Alu.mult)
            den = wk.tile([P, 2, F], bf16, tag="den")
            V.tensor_tensor(den, ar[:, 0:2], _bc(ar[:, 2], 2), op=Alu.add)
            i_den = V.tensor_tensor(den, den, inter, op=Alu.subtract)
            den32 = wk.tile([P, 2, F], f32, tag="den32")
            V.tensor_copy(den32, den)
            rden = wk.tile([P, 2, F], f32, tag="rden")
            V.reciprocal_approx_fast(rden, den32)
            rden16 = wk.tile([P, 2, F], bf16, tag="rden16")
            V.tensor_copy(rden16, rden)
            iou = wk.tile([P, 2, F], bf16, tag="iou")
            i_iou = V.tensor_tensor(iou, inter, rden16, op=Alu.mult)

            # responsibility selection (argmax ties -> box0, like jnp)
            ge = wk.tile([P, F], bf16, tag="ge")
            V.tensor_tensor(ge, iou[:, 0], iou[:, 1], op=Alu.is_ge)
            miou = wk.tile([P, F], bf16, tag="miou")
            i_miou = V.tensor_tensor(miou, iou[:, 0], iou[:, 1], op=Alu.max)
            resp = wk.tile([P, 2, F], bf16, tag="resp")
            V.tensor_tensor(resp[:, 0], ge, conf, op=Alu.mult)
            V.tensor_tensor(resp[:, 1], conf, resp[:, 0], op=Alu.subtract)

            # wh needs sqrt; nm = 1 - conf
            sq = wk.tile([P, 2, 4, F], bf16, tag="sq")
            A.activation(sq[:, 0], W, Act.Sqrt)
            A.activation(sq[:, 1], H, Act.Sqrt)
            nm = wk.tile([P, F], bf16, tag="nm")
            V.tensor_scalar(nm, conf, -1.0, 1.0, op0=Alu.mult, op1=Alu.add)

            # fix dx/dy box1 to slot-matched target, fill dw dh do nb
            V.tensor_tensor(dsq[:, 0, 1], X[:, 1], X[:, 3], op=Alu.subtract)
            V.tensor_tensor(dsq[:, 1, 1], Y[:, 1], Y[:, 3], op=Alu.subtract)
            V.tensor_tensor(dsq[:, 2], sq[:, 0, 0:2], sq[:, 0, 2:4],
                            op=Alu.subtract)
            V.tensor_tensor(dsq[:, 3], sq[:, 1, 0:2], sq[:, 1, 2:4],
                            op=Alu.subtract)
            V.tensor_tensor(dsq[:, 4], pc, _bc(miou, 2), op=Alu.subtract)
            V.tensor_tensor(dsq[:, 5], pc, _bc(nm, 2), op=Alu.mult)
            # mask by responsibility (resp^2 == resp), pipelined with the
            # square+accumulate passes
            V.tensor_tensor(dsq[:, 0:2], dsq[:, 0:2], _bc(resp, 2),
                            op=Alu.mult)
            A.activation(dsq[:, 0:2], dsq[:, 0:2], Act.Square,
                         accum_out=acc[:, 0:1])
            V.tensor_tensor(dsq[:, 2:4], dsq[:, 2:4], _bc(resp, 2),
                            op=Alu.mult)
            A.activation(dsq[:, 2:4], dsq[:, 2:4], Act.Square,
                         accum_out=acc[:, 1:2])
            mo = V.tensor_tensor(dsq[:, 4], dsq[:, 4], resp, op=Alu.mult)
            A.activation(dsq[:, 4], dsq[:, 4], Act.Square,
                         accum_out=acc[:, 2:3])
            A.activation(dsq[:, 5], dsq[:, 5], Act.Square,
                         accum_out=acc[:, 3:4])

            # class: mask by obj then square+accumulate; stagger the masks
            # into the box chain at points where their DMA data has landed.
            # The last quarter's reduction runs on DVE (stt) to shorten the
            # ACT tail.
            gates = [i_den, i_iou, mo, mo]
            trash = wk.tile([P, 20, MQ], bf16, tag="trash")
            for q in range(NCQ):
                cq = conf[:, q * MQ:(q + 1) * MQ]
                mi = V.tensor_tensor(cv[q], cv[q], _bc(cq, 20), op=Alu.mult)
                add_dep_helper(mi.ins, gates[q].ins, sync=False,
                               reason="cls mask staggered into box chain")
                if q >= NCQ - 2:
                    # split the last reductions across DVE and ACT so both
                    # engines finish together
                    V.scalar_tensor_tensor(trash[:, 0:10], cv[q][:, 0:10],
                                           0.0, cv[q][:, 0:10],
                                           op0=Alu.bypass, op1=Alu.mult,
                                           accum_out=acc[:, 4 + q:5 + q])
                    A.activation(cv[q][:, 10:20], cv[q][:, 10:20], Act.Square,
                                 accum_out=acc[:, 8 + q - NCQ + 2:9 + q - NCQ + 2])
                else:
                    A.activation(cv[q], cv[q], Act.Square,
                                 accum_out=acc[:, 4 + q:5 + q])

            nc.sync.dma_start(out.ap(), acc)

    nc.compile()
    return nc


_NC_CACHE = None


def _get_nc():
    global _NC_CACHE
    if _NC_CACHE is None:
        _NC_CACHE = build_nc()
    return _NC_CACHE


# box plane order: (src, channel): X4, W4, Y4, H4, C3
_PLANES = [(0, 0), (0, 5), (1, 0), (1, 5),
           (0, 2), (0, 7), (1, 2), (1, 7),
           (0, 1), (0, 6), (1, 1), (1, 6),
           (0, 3), (0, 8), (1, 3), (1, 8),
           (0, 4), (0, 9), (1, 4)]


def shard_inputs(pred_tensor, target_tensor):
    """Full [16384,7,7,30] f32 -> per-core planar bf16 box + fp8 class."""
    p = np.asarray(pred_tensor, dtype=np.float32).reshape(NCORES, P, F, 30)
    t = np.asarray(target_tensor, dtype=np.float32).reshape(NCORES, P, F, 30)
    src = (p, t)

    box = np.empty((NCORES, P, 19, F), dtype=ml_dtypes.bfloat16)
    for i, (s, ch) in enumerate(_PLANES):
        box[:, :, i] = src[s][..., ch]

    pv = p[..., 10:30].astype(ml_dtypes.float8_e3m4)
    nv = (-t[..., 10:30]).astype(ml_dtypes.float8_e3m4)

    def v_pack(x):  # [NCORES, P, F, 20] -> [NCORES, NCQ, P, 20, MQ]
        y = x.transpose(0, 1, 3, 2).reshape(NCORES, P, 20, NCQ, MQ)
        return np.ascontiguousarray(y.transpose(0, 3, 1, 2, 4))

    cvp, cvn = v_pack(pv), v_pack(nv)
    return [{"box": box[c], "cvp": cvp[c], "cvn": cvn[c]}
            for c in range(NCORES)]


def combine(results):
    """Per-core acc_out [P, NACC] -> 5-tuple of loss scalars."""
    total = np.zeros(5, dtype=np.float64)
    for r in results:
        a = r["acc_out"].astype(np.float64).sum(axis=0)
        total[:4] += a[:4]
        total[4] += a[4:].sum()
    total /= BATCH
    return tuple(np.float32(v) for v in total)


def kernel(pred_tensor, target_tensor):
    nc = _get_nc()
    in_maps = shard_inputs(pred_tensor, target_tensor)
    res = run_bass_kernel_spmd(nc, in_maps, core_ids=list(range(NCORES)))
    return combine(res.results)


# revision 38
# speedup vs baseline: 1.0281x; 1.0281x over previous
"""YOLO-v1-style loss on 8 Trainium2 NeuronCores (Bass/Tile).

Data-parallel over batch: each core gets 2048 of 16384 batch elements
(100,352 cells as 128 partitions x 784 cells); per-partition partial sums
for the 5 loss terms are combined on the host.

Layout: host repacks channels into per-channel planes ([P, plane, cells])
so DVE tensor_tensor ops run dense step-1 bf16 at 2x mode and
tensor_scalar ops at 4x. scalar_tensor_tensor (1x only) is avoided.

IoU via the overlap identity (no corner materialization):
  overlap_x = min(3.5*(pw+tw) - |px-tx|, 7*min(pw,tw)), clamped at 0.

Engine split:
  - DVE: box pipeline (2x-mode bf16 tensor_tensor) + obj/resp masks
  - ACT: sqrt/abs, square+accumulate reduction passes
  - Pool: SWDGE descriptor generation only (its tensor ops are slow and
    fight DVE for SBUF ports)
  - DMA CCE: class diff (p - t) computed inline in the DMA engines:
    p streams in with an fp8->bf16 cast, host-negated t accumulates with
    cce add. CCE accumulates silently corrupt beyond 2048 elements per
    partition per DMA, so the accumulate is issued in <=1960-element calls.

add_dep_helper sync=False edges steer the Tile list scheduler: the class
masks slot into the box chain where their DMA data has landed, and the
class stream is throttled behind the box stream for HBM priority.

Self-contained: hardcodes all shapes; needs numpy + ml_dtypes + concourse.
"""

import numpy as np
import ml_dtypes

import concourse.bass as bass
import concourse.bacc as bacc
import concourse.tile as tile
import concourse.mybir as mybir
from concourse.bass_utils import run_bass_kernel_spmd
from bass_rust import add_dep_helper

f32 = mybir.dt.float32
bf16 = mybir.dt.bfloat16
f8e3 = mybir.dt.float8e3
Alu = mybir.AluOpType
Act = mybir.ActivationFunctionType

S = 7
BATCH = 16384
NCORES = 8
PER = BATCH // NCORES          # 2048 batch elems per core
P = 128                        # partitions
F = PER * S * S // P           # 784 cells per partition
NCQ = 4                        # class cell chunks
MQ = F // NCQ                  # 196
HS = S / 2.0                   # 3.5
S2 = float(S * S)              # 49

NACC = 4 + NCQ + 2


def _bc(x, r):
    """[P, ...] -> [P, r, ...]: broadcast (step-0) over a new outer dim."""
    return bass.AP(tensor=x.tensor, offset=x.offset,
                   ap=[x.ap[0], [0, r]] + list(x.ap[1:]))


def build_nc():
    nc = bacc.Bacc("TRN2", target_bir_lowering=False, debug=False,
                   num_devices=NCORES)
    # box planes (bf16) [P, 19, F]:
    #   0-3  X4 = px0 px1 tx0 tx1      4-7  W4 = pw0 pw1 tw0 tw1
    #   8-11 Y4 = py0 py1 ty0 ty1     12-15 H4 = ph0 ph1 th0 th1
    #   16-18 C3 = pc0 pc1 conf
    box = nc.dram_tensor("box", [P, 19, F], bf16, kind="ExternalInput")
    # class streams (fp8 e3m4): pred and negated target, channel-major
    cvp = nc.dram_tensor("cvp", [NCQ, P, 20, MQ], f8e3, kind="ExternalInput")
    cvn = nc.dram_tensor("cvn", [NCQ, P, 20, MQ], f8e3, kind="ExternalInput")

    out = nc.dram_tensor("acc_out", [P, NACC], f32, kind="ExternalOutput")

    V = nc.vector
    A = nc.scalar
    G = nc.gpsimd

    with tile.TileContext(nc) as tc:
        with (
            tc.tile_pool(name="inp", bufs=1) as inp,
            tc.tile_pool(name="cls", bufs=4) as clsb,
            tc.tile_pool(name="wk", bufs=1) as wk,
            tc.tile_pool(name="one", bufs=1) as one,
        ):
            acc = one.tile([P, NACC], f32)
            V.memset(acc, 0.0)

            # ---- box DMAs first: V work starts as soon as X4 lands ----
            bxa = inp.tile([P, 8, F], bf16, tag="bxa")   # X4 W4
            nc.sync.dma_start(bxa[:, 0:4], box.ap()[:, 0:4])
            i_w4 = nc.sync.dma_start(bxa[:, 4:8], box.ap()[:, 4:8])
            bxb = inp.tile([P, 8, F], bf16, tag="bxb")   # Y4 H4
            nc.sync.dma_start(bxb[:, 0:4], box.ap()[:, 8:12])
            i_h4 = nc.sync.dma_start(bxb[:, 4:8], box.ap()[:, 12:16])
            bxc = inp.tile([P, 3, F], bf16, tag="bxc")   # C3
            i_c3 = nc.sync.dma_start(bxc, box.ap()[:, 16:19])
            box_gate = [i_w4, i_h4, i_c3, i_c3]

            # ---- class diff stream: SWDGE cast + CCE add (d = p - t) ----
            # CCE accumulate caps at 2048 elems/partition per DMA: the
            # cast moves all 20 channels, the accumulate goes in 2 halves.
            cv = []
            for q in range(NCQ):
                dv = clsb.tile([P, 20, MQ], bf16, tag="dv")
                ic = G.dma_start(dv, cvp.ap()[q])
                # throttle class stream behind the box DMAs so the box
                # pipeline's inputs land at full HBM rate first
                add_dep_helper(ic.ins, box_gate[q].ins,
                               reason="cls stream after box stream")
                G.dma_start(dv[:, 0:10], cvn.ap()[q][:, 0:10],
                            accum_op=Alu.add)
                G.dma_start(dv[:, 10:20], cvn.ap()[q][:, 10:20],
                            accum_op=Alu.add)
                cv.append(dv)

            X, W = bxa[:, 0:4], bxa[:, 4:8]
            Y, H = bxb[:, 0:4], bxb[:, 4:8]
            pc = bxc[:, 0:2]
            conf = bxc[:, 2]

            # squared-loss ingredients [dx(2) dy(2) dw(2) dh(2) do(2) nb(2)]
            dsq = wk.tile([P, 6, 2, F], bf16, tag="dsq")

            # dx_b = px_b - tx0 (slot-0 target; dsq[0,1] re-done per slot later)
            V.tensor_tensor(dsq[:, 0], X[:, 0:2], _bc(X[:, 2], 2),
                            op=Alu.subtract)
            V.tensor_tensor(dsq[:, 1], Y[:, 0:2], _bc(Y[:, 2], 2),
                            op=Alu.subtract)
            adx = wk.tile([P, 2, 2, F], bf16, tag="adx")
            A.activation(adx[:, 0], dsq[:, 0], Act.Abs)
            A.activation(adx[:, 1], dsq[:, 1], Act.Abs)

            # u = 3.5*(pw_b + tw0) - |dx|
            u = wk.tile([P, 2, 2, F], bf16, tag="u")
            V.tensor_tensor(u[:, 0], W[:, 0:2], _bc(W[:, 2], 2), op=Alu.add)
            V.tensor_tensor(u[:, 1], H[:, 0:2], _bc(H[:, 2], 2), op=Alu.add)
            V.tensor_scalar(u, u, HS, None, op0=Alu.mult)
            V.tensor_tensor(u, u, adx, op=Alu.subtract)

            # m7 = 7 * min(pw_b, tw0); overlap = relu(min(u, m7))
            m7 = wk.tile([P, 2, 2, F], bf16, tag="m7")
            V.tensor_tensor(m7[:, 0], W[:, 0:2], _bc(W[:, 2], 2), op=Alu.min)
            V.tensor_tensor(m7[:, 1], H[:, 0:2], _bc(H[:, 2], 2), op=Alu.min)
            V.tensor_scalar(m7, m7, float(S), None, op0=Alu.mult)
            whr = wk.tile([P, 2, 2, F], bf16, tag="whr")
            V.tensor_tensor(whr, u, m7, op=Alu.min)
            V.tensor_scalar(whr, whr, 0.0, None, op0=Alu.max)

            inter = wk.tile([P, 2, F], bf16, tag="inter")
            V.tensor_tensor(inter, whr[:, 0], whr[:, 1], op=Alu.mult)

            # areas (x S^2): [ap0 ap1 at]
            ar = wk.tile([P, 3, F], bf16, tag="ar")
            V.tensor_scalar(ar, W[:, 0:3], S2, None, op0=Alu.mult)
            V.tensor_tensor(ar, ar, H[:, 0:3], op=Alu.mult)
            den = wk.tile([P, 2, F], bf16, tag="den")
            V.tensor_tensor(den, ar[:, 0:2], _bc(ar[:, 2], 2), op=Alu.add)
            i_den = V.tensor_tensor(den, den, inter, op=Alu.subtract)
            den32 = wk.tile([P, 2, F], f32, tag="den32")
            V.tensor_copy(den32, den)
            rden = wk.tile([P, 2, F], f32, tag="rden")
            V.reciprocal_approx_fast(rden, den32)
            rden16 = wk.tile([P, 2, F], bf16, tag="rden16")
            V.tensor_copy(rden16, rden)
            iou = wk.tile([P, 2, F], bf16, tag="iou")
            i_iou = V.tensor_tensor(iou, inter, rden16, op=Alu.mult)

            # responsibility selection (argmax ties -> box0, like jnp)
            ge = wk.tile([P, F], bf16, tag="ge")
            V.tensor_tensor(ge, iou[:, 0], iou[:, 1], op=Alu.is_ge)
            miou = wk.tile([P, F], bf16, tag="miou")
            i_miou = V.tensor_tensor(miou, iou[:, 0], iou[:, 1], op=Alu.max)
            resp = wk.tile([P, 2, F], bf16, tag="resp")
            V.tensor_tensor(resp[:, 0], ge, conf, op=Alu.mult)
            V.tensor_tensor(resp[:, 1], conf, resp[:, 0], op=Alu.subtract)

            # wh needs sqrt; nm = 1 - conf
            sq = wk.tile([P, 2, 4, F], bf16, tag="sq")
            A.activation(sq[:, 0], W, Act.Sqrt)
            A.activation(sq[:, 1], H, Act.Sqrt)
            nm = wk.tile([P, F], bf16, tag="nm")
            V.tensor_scalar(nm, conf, -1.0, 1.0, op0=Alu.mult, op1=Alu.add)

            # fix dx/dy box1 to slot-matched target, fill dw dh do nb
            V.tensor_tensor(dsq[:, 0, 1], X[:, 1], X[:, 3], op=Alu.subtract)
            V.tensor_tensor(dsq[:, 1, 1], Y[:, 1], Y[:, 3], op=Alu.subtract)
            V.tensor_tensor(dsq[:, 2], sq[:, 0, 0:2], sq[:, 0, 2:4],
                            op=Alu.subtract)
            V.tensor_tensor(dsq[:, 3], sq[:, 1, 0:2], sq[:, 1, 2:4],
                            op=Alu.subtract)
            V.tensor_tensor(dsq[:, 4], pc, _bc(miou, 2), op=Alu.subtract)
            V.tensor_tensor(dsq[:, 5], pc, _bc(nm, 2), op=Alu.mult)
            # mask by responsibility (resp^2 == resp), pipelined with the
            # square+accumulate passes
            V.tensor_tensor(dsq[:, 0:2], dsq[:, 0:2], _bc(resp, 2),
                            op=Alu.mult)
            A.activation(dsq[:, 0:2], dsq[:, 0:2], Act.Square,
                         accum_out=acc[:, 0:1])
            V.tensor_tensor(dsq[:, 2:4], dsq[:, 2:4], _bc(resp, 2),
                            op=Alu.mult)
            A.activation(dsq[:, 2:4], dsq[:, 2:4], Act.Square,
                         accum_out=acc[:, 1:2])
            mo = V.tensor_tensor(dsq[:, 4], dsq[:, 4], resp, op=Alu.mult)
            A.activation(dsq[:, 4], dsq[:, 4], Act.Square,
                         accum_out=acc[:, 2:3])
            A.activation(dsq[:, 5], dsq[:, 5], Act.Square,
                         accum_out=acc[:, 3:4])

            # class: mask by obj then square+accumulate; stagger the masks
            # into the box chain at points where their DMA data has landed.
            # The last quarter's reduction runs on DVE (stt) to shorten the
            # ACT tail.
            gates = [i_den, i_iou, mo, mo]
            trash = wk.tile([P, 20, MQ], bf16, tag="trash")
            for q in range(NCQ):
                cq = conf[:, q * MQ:(q + 1) * MQ]
                mi = V.tensor_tensor(cv[q], cv[q], _bc(cq, 20), op=Alu.mult)
                add_dep_helper(mi.ins, gates[q].ins, sync=False,
                               reason="cls mask staggered into box chain")
                if q >= NCQ - 2:
                    # split the last reductions across DVE and ACT so both
                    # engines finish together
                    V.scalar_tensor_tensor(trash[:, 0:10], cv[q][:, 0:10],
                                           0.0, cv[q][:, 0:10],
                                           op0=Alu.bypass, op1=Alu.mult,
                                           accum_out=acc[:, 4 + q:5 + q])
                    A.activation(cv[q][:, 10:20], cv[q][:, 10:20], Act.Square,
                                 accum_out=acc[:, 8 + q - NCQ + 2:9 + q - NCQ + 2])
                else:
                    A.activation(cv[q], cv[q], Act.Square,
                                 accum_out=acc[:, 4 + q:5 + q])

            nc.sync.dma_start(out.ap(), acc)

    nc.compile()
    return nc


_NC_CACHE = None


def _get_nc():
    global _NC_CACHE
    if _NC_CACHE is None:
        _NC_CACHE = build_nc()
    return _NC_CACHE


# box plane order: (src, channel): X4, W4, Y4, H4, C3
_PLANES = [(0, 0), (0, 5), (1, 0), (1, 5),
           (0, 2), (0, 7), (1, 2), (1, 7),
           (0, 1), (0, 6), (1, 1), (1, 6),
           (0, 3), (0, 8), (1, 3), (1, 8),
           (0, 4), (0, 9), (1, 4)]


def shard_inputs(pred_tensor, target_tensor):
    """Full [16384,7,7,30] f32 -> per-core planar bf16 box + fp8 class."""
    p = np.asarray(pred_tensor, dtype=np.float32).reshape(NCORES, P, F, 30)
    t = np.asarray(target_tensor, dtype=np.float32).reshape(NCORES, P, F, 30)
    src = (p, t)

    box = np.empty((NCORES, P, 19, F), dtype=ml_dtypes.bfloat16)
    for i, (s, ch) in enumerate(_PLANES):
        box[:, :, i] = src[s][..., ch]

    pv = p[..., 10:30].astype(ml_dtypes.float8_e3m4)
    nv = (-t[..., 10:30]).astype(ml_dtypes.float8_e3m4)

    def v_pack(x):  # [NCORES, P, F, 20] -> [NCORES, NCQ, P, 20, MQ]
        y = x.transpose(0, 1, 3, 2).reshape(NCORES, P, 20, NCQ, MQ)
        return np.ascontiguousarray(y.transpose(0, 3, 1, 2, 4))

    cvp, cvn = v_pack(pv), v_pack(nv)
    return [{"box": box[c], "cvp": cvp[c], "cvn": cvn[c]}
            for c in range(NCORES)]


def combine(results):
    """Per-core acc_out [P, NACC] -> 5-tuple of loss scalars."""
    total = np.zeros(5, dtype=np.float64)
    for r in results:
        a = r["acc_out"].astype(np.float64).sum(axis=0)
        total[:4] += a[:4]
        total[4] += a[4:].sum()
    total /= BATCH
    return tuple(np.float32(v) for v in total)


def kernel(pred_tensor, target_tensor):
    nc = _get_nc()
    in_maps = shard_inputs(pred_tensor, target_tensor)
    res = run_bass_kernel_spmd(nc, in_maps, core_ids=list(range(NCORES)))
    return combine(res.results)
